# revision 1
# baseline (speedup 1.0000x reference)
"""Transformer decoder layer (causal self-attn + cross-attn + FFN, 3 post-LNs)
on 8 Trainium2 NeuronCores — token-parallel version, zero collectives.

Sharding: 2-way data parallel (batch) x 4-way query-token striping.
  core c: batch g = c // 4, stripe r = c % 4 owns the 128-row blocks
  {r, r+4, r+8, r+12} of the sequence (512 query tokens).
  - K/V are computed redundantly on every core from the full input /
    encoder_output (which each core holds) for all 16 heads.
  - out-projections and the FFN are complete per token -> no reductions.
  - causality is data-driven (cmask4 per core), so the instruction
    stream is identical on all cores (true SPMD).

On-chip layouts (per core):
  x0T/encT  [128, 8, 2048]  bf16   feature-major full activations
  xrowT     [128, 8, 512]   bf16   feature-major own-token activations
  kT        [128, 8, 2048]  bf16   head-dim on partitions (2 heads x 64)
  qT        [128, 8, 512]   bf16
  v         [128, 16, 16, 65] fp8  token-major V (+ ones col = rowsum)
  at        [128, 16, 512]  fp8    exp(scores), k-major
  poT       [65, 512] PSUM  f32    v.T @ at (row 64 = softmax denom Z)
  oT        [128, 8, 512]   bf16   normalized attention out, feature-major

Softmax normalization: rcp(Z) row broadcast down 64 partitions via a
rank-1 matmul (ones[1,64].T @ rz[1,512]), then one DVE multiply.
"""

import numpy as np
import ml_dtypes

import concourse.bass as bass
import concourse.bacc as bacc
import concourse.tile as tile
from concourse import mybir
from concourse import bass_utils
from concourse.masks import make_identity

F32 = mybir.dt.float32
BF16 = mybir.dt.bfloat16
FP8 = mybir.dt.float8e4
AF = mybir.ActivationFunctionType
ALU = mybir.AluOpType

E = 1024
EB = 8           # E / 128
H = 16
HP = 8           # head pairs
DK = 64
S = 2048
TB = 16          # full-token 128-blocks
TBQ = 4          # own-token 128-blocks
SQ = 512         # own query tokens


def _ts(i, n):
    return slice(i * n, (i + 1) * n)


def _pbcast(ap, p=128):
    """Broadcast a 1D DRAM AP across p partitions (partition step 0)."""
    return bass.AP(tensor=ap.tensor, offset=ap.offset, ap=[[0, p]] + list(ap.ap))


PHASES = ["null", "x0t", "saqkv", "saattn", "ln1", "cakv", "caattn",
          "ln2", "ffn1", "ffn2", "full"]


def build_decoder_nc(S_arg: int = S, num_devices: int = 8,
                     stop_after: str | None = None, reps: int = 1):
    assert S_arg == S
    nc = bacc.Bacc("TRN2", target_bir_lowering=False, debug=False,
                   num_devices=num_devices)

    din = {}

    def inp(name, shape, dt):
        din[name] = nc.dram_tensor(name, list(shape), dt, kind="ExternalInput")
        return din[name]

    inp("x0T_b", [E, S], BF16)           # input (batch g), host-transposed
    inp("x0rT_b", [E, SQ], BF16)         # own stripes, host-transposed
    inp("x0res", [SQ, E], F32)           # own stripes + sa_bo (residual)
    inp("encT_b", [E, S], BF16)          # encoder out, host-transposed
    for p in ("sa", "ca"):
        inp(f"{p}_wkv", [E, 2 * E], BF16)    # [wk | wv]
        inp(f"{p}_wq", [E, E], BF16)
        inp(f"{p}_wo", [E, E], BF16)
        inp(f"{p}_bq", [E], F32)
        inp(f"{p}_bk", [E], F32)
        inp(f"{p}_bv", [E], BF16)        # bcast-loaded
    inp("ca_bo", [E], BF16)
    inp("w1", [E, 4 * E], BF16)
    inp("b1", [4 * E], F32)
    inp("w2", [4 * E, E], BF16)
    inp("b2", [E], BF16)
    for i in (1, 2, 3):
        inp(f"ln{i}_g", [E], BF16)
        inp(f"ln{i}_b", [E], BF16)
    inp("cmask4", [4, 128, 128], FP8)    # per-core causal block masks

    out = nc.dram_tensor("out", [SQ, E], F32, kind="ExternalOutput")

    with tile.TileContext(nc) as tc:
        _emit(tc, din, out, stop_after, reps=reps)

    nc.compile()
    return nc


def _emit(tc, din, out, stop_after=None, reps=1):
    nc = tc.nc

    def cut(phase):
        return stop_after == phase

    def finish():
        nc.sync.dma_start(out=out.ap(), in_=din["x0res"].ap())

    from contextlib import ExitStack
    with ExitStack() as _es:
        _es.enter_context(nc.allow_low_precision(
            reason="fp8 attn weights / bf16 softmax rcp; validated by rel-err"))
        def _pool(**kw):
            return _es.enter_context(tc.tile_pool(**kw))
        const = _pool(name="const", bufs=1)
        xt_pool = _pool(name="xt", bufs=1)          # x0T / encT
        wkv_pool = _pool(name="wkv", bufs=1)        # wkv / w1-half
        wq_pool = _pool(name="wq", bufs=1)          # wq / wo
        kt_pool = _pool(name="kt", bufs=1)          # kT / w2-half
        v_pool = _pool(name="vp", bufs=1)           # v / ffn y
        qt_pool = _pool(name="qt", bufs=1)
        attn_pool = _pool(name="att", bufs=1)       # at / hT
        o_pool = _pool(name="ot", bufs=1)
        xrt_pool = _pool(name="xrt", bufs=1)        # xrowT
        res_pool = _pool(name="res", bufs=1)        # x1/x2 row bf16
        lnp = _pool(name="lnp", bufs=2)
        lnr = _pool(name="lnr", bufs=1)
        zp = _pool(name="zp", bufs=2)
        stat = _pool(name="stat", bufs=8)
        mmp = _pool(name="mmp", bufs=2, space="PSUM")
        po_pool = _pool(name="po", bufs=2, space="PSUM")
        aux = _pool(name="aux", bufs=2, space="PSUM")
        # ---------------- constants ----------------
        ident = const.tile([128, 128], BF16)
        make_identity(nc, ident)
        eps_t = const.tile([128, 1], F32)
        nc.vector.memset(eps_t, 1e-12)
        ones64 = const.tile([1, 64], BF16)
        nc.vector.memset(ones64, 1.0)
        cmask4 = const.tile([128, 4, 128], FP8)
        nc.sync.dma_start(out=cmask4,
                          in_=din["cmask4"].ap().rearrange("i p q -> p i q"))

        _bc = {}

        def bcast(name, tag=None, width=E):
            if name not in _bc:
                t = const.tile([128, width], BF16, name=f"bc_{name}",
                               tag=tag or f"bc_{name}")
                nc.sync.dma_start(out=t, in_=_pbcast(din[name].ap()))
                _bc[name] = t
            return _bc[name]

        def pp_bias(name, nj, dt=F32):
            t = const.tile([128, nj], dt, name=f"ppb_{name}")
            nc.sync.dma_start(out=t,
                              in_=din[name].ap().rearrange("(j p) -> p j", p=128))
            return t

        bq = {p: pp_bias(f"{p}_bq", 8) for p in ("sa", "ca")}
        bk = {p: pp_bias(f"{p}_bk", 8) for p in ("sa", "ca")}
        b1_t = pp_bias("b1", 32)

        # ---------------- shared helpers ----------------
        def load_wkv(pref):
            t = wkv_pool.tile([128, EB, 2 * E], BF16, tag="wkv",
                              name=f"{pref}_wkv")
            nc.sync.dma_start(out=t, in_=din[f"{pref}_wkv"].ap().rearrange(
                "(eb p) m -> p eb m", p=128))
            return t

        def load_sq(name, tag="wq"):
            t = wq_pool.tile([128, EB, E], BF16, tag=tag, name=f"{name}_sb")
            nc.sync.dma_start(out=t, in_=din[name].ap().rearrange(
                "(eb p) m -> p eb m", p=128))
            return t

        def kv_proj(xT, wkv, bkt, bvb, kT, v):
            # kT [128, hp, S] bf16 ; v [128, tb, h, 65] fp8
            for hp in range(HP):
                for t2 in range(2):
                    mm = mmp.tile([128, 1024], F32, tag="mm")
                    for i in range(2):
                        tt = 2 * t2 + i
                        for eb in range(EB):
                            nc.tensor.matmul(mm[:, _ts(i, 512)],
                                             wkv[:, eb, _ts(hp, 128)],
                                             xT[:, eb, _ts(tt, 512)],
                                             start=(eb == 0), stop=(eb == EB - 1))
                    nc.vector.tensor_scalar_add(kT[:, hp, _ts(t2, 1024)], mm,
                                                bkt[:, hp:hp + 1])
            for tb in range(TB):
                mm = mmp.tile([128, 1024], F32, tag="mm")
                for half in range(2):
                    for eb in range(EB):
                        nc.tensor.matmul(mm[:, _ts(half, 512)],
                                         xT[:, eb, _ts(tb, 128)],
                                         wkv[:, eb, E + half * 512:
                                             E + half * 512 + 512],
                                         start=(eb == 0), stop=(eb == EB - 1))
                nc.vector.tensor_add(
                    v[:, tb, :, 0:64],
                    mm.rearrange("p (h d) -> p h d", d=64),
                    bvb.rearrange("p (h d) -> p h d", d=64))

        def q_proj(xrT, wq, bqt, qT):
            for hp2 in range(HP // 2):
                mm = mmp.tile([128, 1024], F32, tag="mm")
                for i in range(2):
                    hp = 2 * hp2 + i
                    for eb in range(EB):
                        nc.tensor.matmul(mm[:, _ts(i, 512)],
                                         wq[:, eb, _ts(hp, 128)],
                                         xrT[:, eb, :],
                                         start=(eb == 0), stop=(eb == EB - 1))
                for i in range(2):
                    nc.vector.tensor_scalar_add(qT[:, 2 * hp2 + i, :],
                                                mm[:, _ts(i, 512)],
                                                bqt[:, 2 * hp2 + i:2 * hp2 + i + 1])

        def attention(qT, kT, v, oT, causal):
            for hp in range(HP):
                at = attn_pool.tile([128, TB, 2, 512], FP8, tag="at")
                for kb in range(TB):
                    c0 = 128 * (kb // 4) if causal else 0
                    mm = mmp.tile([128, 1024], F32, tag="mm")
                    mm2 = mm.rearrange("p (i q) -> p i q", i=2)
                    nc.tensor.matmul(mm[:, c0:512], kT[0:64, hp, _ts(kb, 128)],
                                     qT[0:64, hp, c0:], start=True, stop=True,
                                     tile_position=(0, 0))
                    nc.tensor.matmul(mm[:, 512 + c0:], kT[64:128, hp, _ts(kb, 128)],
                                     qT[64:128, hp, c0:], start=True, stop=True,
                                     tile_position=(64, 0))
                    nc.scalar.activation(at[:, kb, :, c0:], mm2[:, :, c0:],
                                         AF.Exp, scale=0.125)
                    if causal:
                        cm = cmask4[:, kb % 4, :]
                        cm2 = bass.AP(tensor=cm.tensor, offset=cm.offset,
                                      ap=[list(cm.ap)[0], [0, 2],
                                          list(cm.ap)[1]])
                        nc.vector.tensor_mul(at[:, kb, :, c0:c0 + 128],
                                             at[:, kb, :, c0:c0 + 128], cm2)
                for h2 in range(2):
                    hs = slice(h2 * 64, h2 * 64 + 64)
                    pot = po_pool.tile([128, 512], F32, tag="po")
                    for kb in range(TB):
                        c0 = 128 * (kb // 4) if causal else 0
                        nc.tensor.matmul(pot[0:65, c0:], v[:, kb, 2 * hp + h2, :],
                                         at[:, kb, h2, c0:],
                                         start=(kb == 0), stop=(kb == TB - 1),
                                         skip_group_check=True)
                    rzb = zp.tile([1, 512], BF16, tag="zb")
                    nc.vector.reciprocal(rzb, pot[64:65, :])
                    pb = aux.tile([128, 512], F32, tag="aux")
                    nc.tensor.matmul(pb[0:64, :], ones64, rzb,
                                     start=True, stop=True)
                    pbs = zp.tile([64, 512], BF16, tag="pbs")
                    nc.vector.tensor_copy(pbs, pb[0:64, :])
                    nc.vector.tensor_tensor(oT[hs, hp, :], pot[0:64, :],
                                            pbs, ALU.mult)

        def layer_norm_block(ld, i, out_bf, tb):
            # in-place LN of ld [128, E]; writes bf16 copy to out_bf[:, tb, :]
            st = stat.tile([128, 2, 6], F32, tag="bnst")
            for sg in range(2):
                nc.vector.bn_stats(st[:, sg, :], ld[:, _ts(sg, 512)])
            mv = stat.tile([128, 2], F32, tag="bnmv")
            nc.vector.bn_aggr(mv, st)
            sd = stat.tile([128, 1], F32, tag="sd")
            nc.scalar.activation(sd, mv[:, 1:2], AF.Sqrt, bias=eps_t)
            rstd = stat.tile([128, 1], F32, tag="rstd")
            nc.vector.reciprocal(rstd, sd)
            nc.vector.tensor_scalar(ld, ld, mv[:, 0:1], rstd,
                                    ALU.subtract, ALU.mult)
            nc.vector.tensor_mul(ld, ld, bcast(f"ln{i}_g"))
            if out_bf is not None:
                nc.vector.tensor_add(out_bf[:, tb, :], ld, bcast(f"ln{i}_b"))
            return ld

        def out_proj_ln(oT, wo, i, residual, out_bf):
            # y = oT.T @ wo (+ residual [+ca_bo]) -> LN_i -> out_bf bf16
            for tb in range(TBQ):
                ld = lnp.tile([128, E], F32, tag="ln_io")
                if i == 1:
                    res = lnr.tile([128, E], F32, tag="ln_res")
                    nc.gpsimd.dma_start(out=res,
                                        in_=din["x0res"].ap()[_ts(tb, 128), :])
                mm = mmp.tile([128, 1024], F32, tag="mm")
                for ns in range(2):
                    for jb in range(EB):
                        nc.tensor.matmul(mm[:, _ts(ns, 512)],
                                         oT[:, jb, _ts(tb, 128)],
                                         wo[:, jb, _ts(ns, 512)],
                                         start=(jb == 0), stop=(jb == EB - 1))
                if i == 1:
                    nc.vector.tensor_add(ld, mm, res)
                else:
                    nc.vector.scalar_tensor_tensor(
                        ld, mm, 1.0, bcast("ca_bo", tag="bob"),
                        ALU.mult, ALU.add)
                    nc.vector.tensor_add(ld, ld, residual[:, tb, :])
                layer_norm_block(ld, i, out_bf, tb)

        def row_transpose(src_bf, dst_T):
            # src [128, TBQ, E] bf16 token-major -> dst [128, EB, SQ]
            for tb in range(TBQ):
                for eb in range(EB):
                    pt = aux.tile([128, 512], BF16, tag="aux")
                    nc.tensor.transpose(pt[:, 0:128], src_bf[:, tb, _ts(eb, 128)],
                                        ident)
                    nc.vector.tensor_copy(dst_T[:, eb, _ts(tb, 128)], pt[:, 0:128])

        # ================= start =================
        if cut("null"):
            finish()
            return
        for _rep in range(reps):

            x0T = xt_pool.tile([128, EB, S], BF16, tag="xT", name="x0T")
            nc.scalar.dma_start(out=x0T, in_=din["x0T_b"].ap().rearrange(
                "(eb p) t -> p eb t", p=128))
            x0rT = xrt_pool.tile([128, EB, SQ], BF16, tag="xrT", name="x0rT")
            nc.sync.dma_start(out=x0rT, in_=din["x0rT_b"].ap().rearrange(
                "(eb p) t -> p eb t", p=128))

            if cut("x0t"):
                finish()
                return

            sa_wkv = load_wkv("sa")
            sa_wq = load_sq("sa_wq")

            kT = kt_pool.tile([128, HP, S], BF16, tag="kT", name="sa_kT")
            v = v_pool.tile([128, TB, H, 65], FP8, tag="v", name="sa_v")
            nc.vector.memset(v[:, :, :, 64:65], 1.0)
            qT = qt_pool.tile([128, HP, SQ], BF16, tag="qT", name="sa_qT")

            kv_proj(x0T, sa_wkv, bk["sa"], bcast("sa_bv", tag="bv"), kT, v)
            q_proj(x0rT, sa_wq, bq["sa"], qT)

            # loads that overlap SA attention (slots freed by the projections)
            sa_wo = load_sq("sa_wo")
            encT = xt_pool.tile([128, EB, S], BF16, tag="xT", name="encT")
            nc.scalar.dma_start(out=encT, in_=din["encT_b"].ap().rearrange(
                "(eb p) t -> p eb t", p=128))
            ca_wkv = load_wkv("ca")

            if cut("saqkv"):
                finish()
                return

            oT = o_pool.tile([128, HP, SQ], FP8, tag="oT", name="sa_oT")
            attention(qT, kT, v, oT, causal=True)

            if cut("saattn"):
                finish()
                return

            x1row_bf = res_pool.tile([128, TBQ, E], BF16, tag="res", name="xrow_bf")
            out_proj_ln(oT, sa_wo, 1, None, x1row_bf)

            if cut("ln1"):
                finish()
                return

            # ================= cross-attention =================
            x1rT = xrt_pool.tile([128, EB, SQ], BF16, tag="xrT", name="x1rT")
            row_transpose(x1row_bf, x1rT)

            ca_wq = load_sq("ca_wq")
            ca_kT = kt_pool.tile([128, HP, S], BF16, tag="kT", name="ca_kT")
            ca_v = v_pool.tile([128, TB, H, 65], FP8, tag="v", name="ca_v")
            nc.vector.memset(ca_v[:, :, :, 64:65], 1.0)
            ca_qT = qt_pool.tile([128, HP, SQ], BF16, tag="qT", name="ca_qT")

            kv_proj(encT, ca_wkv, bk["ca"], bcast("ca_bv", tag="bv"), ca_kT, ca_v)
            q_proj(x1rT, ca_wq, bq["ca"], ca_qT)

            ca_wo = load_sq("ca_wo")

            if cut("cakv"):
                finish()
                return

            ca_oT = o_pool.tile([128, HP, SQ], FP8, tag="oT", name="ca_oT")
            attention(ca_qT, ca_kT, ca_v, ca_oT, causal=False)

            # FFN pass-0 weight prefetch (overlaps CA out-proj/LN2)
            w1ap = din["w1"].ap().rearrange("(eb p) m -> p eb m", p=128)
            w1p0 = wkv_pool.tile([128, EB, 2 * E], BF16, tag="wkv", name="w1_0")
            nc.sync.dma_start(out=w1p0, in_=w1ap[:, :, _ts(0, 2 * E)])
            w2p0 = kt_pool.tile([128, 16, E], BF16, tag="kT", name="w2_0")
            nc.sync.dma_start(
                out=w2p0,
                in_=din["w2"].ap()[_ts(0, 2 * E), :].rearrange(
                    "(fb q) n -> q fb n", q=128))

            if cut("caattn"):
                finish()
                return

            out_proj_ln(ca_oT, ca_wo, 2, x1row_bf, x1row_bf)  # x2row overwrites

            if cut("ln2"):
                finish()
                return

            # ================= FFN =================
            x2rT = xrt_pool.tile([128, EB, SQ], BF16, tag="xrT", name="x2rT")
            row_transpose(x1row_bf, x2rT)

            y = v_pool.tile([128, TBQ, E], F32, tag="v", name="ffn_y")
            for p in range(2):
                if p == 0:
                    w1p, w2p = w1p0, w2p0
                else:
                    w1p = wkv_pool.tile([128, EB, 2 * E], BF16, tag="wkv",
                                        name=f"w1_{p}")
                    nc.sync.dma_start(out=w1p, in_=w1ap[:, :, _ts(p, 2 * E)])
                    w2p = kt_pool.tile([128, 16, E], BF16, tag="kT",
                                       name=f"w2_{p}")
                    nc.sync.dma_start(
                        out=w2p,
                        in_=din["w2"].ap()[_ts(p, 2 * E), :].rearrange(
                            "(fb q) n -> q fb n", q=128))
                hT = attn_pool.tile([128, 16, 512], FP8, tag="at", name=f"hT_{p}")
                for hb2 in range(8):
                    mm = mmp.tile([128, 1024], F32, tag="mm")
                    for i in range(2):
                        hb = 2 * hb2 + i
                        for eb in range(EB):
                            nc.tensor.matmul(mm[:, _ts(i, 512)],
                                             w1p[:, eb, _ts(hb, 128)],
                                             x2rT[:, eb, :],
                                             start=(eb == 0), stop=(eb == EB - 1))
                    for i in range(2):
                        hb = 2 * hb2 + i
                        gfb = p * 16 + hb
                        nc.scalar.activation(hT[:, hb, :], mm[:, _ts(i, 512)],
                                             AF.Relu, bias=b1_t[:, gfb:gfb + 1])
                if cut("ffn1") and p == 0:
                    finish()
                    return
                for tb in range(TBQ):
                    mm = mmp.tile([128, 1024], F32, tag="mm")
                    for ns in range(2):
                        for fb in range(16):
                            nc.tensor.matmul(mm[:, _ts(ns, 512)],
                                             hT[:, fb, _ts(tb, 128)],
                                             w2p[:, fb, _ts(ns, 512)],
                                             start=(fb == 0), stop=(fb == 15))
                    if p == 0:
                        nc.vector.scalar_tensor_tensor(
                            y[:, tb, :], mm, 1.0, bcast("b2", tag="bob"),
                            ALU.mult, ALU.add)
                    else:
                        ld = lnp.tile([128, E], F32, tag="ln_io")
                        nc.vector.tensor_tensor(ld, y[:, tb, :], mm, ALU.add)
                        nc.vector.tensor_add(ld, ld, x1row_bf[:, tb, :])
                        layer_norm_block(ld, 3, None, tb)
                        nc.vector.tensor_add(ld, ld, bcast("ln3_b"))
                        nc.sync.dma_start(out=out.ap()[_ts(tb, 128), :], in_=ld)

        if cut("ffn2"):
            return


# ====================== host side ======================

def stripe_idx(r):
    blocks = [r, r + 4, r + 8, r + 12]
    return np.concatenate([np.arange(128 * b, 128 * b + 128) for b in blocks])


def make_cmask4(r):
    # i < r: pass; i == r: lower-tri straddle; i > r: blocked
    m = np.zeros((4, 128, 128), dtype=np.float32)
    pk = np.arange(128)[:, None]
    pq = np.arange(128)[None, :]
    for i in range(4):
        if i < r:
            m[i] = 1.0
        elif i == r:
            m[i] = (pk <= pq).astype(np.float32)
    return m.astype(ml_dtypes.float8_e4m3fn)


def shard_inputs(inputs, num_devices=8):
    bf = ml_dtypes.bfloat16
    f32 = np.float32
    inp = {k: np.asarray(v) for k, v in inputs.items()}
    in_maps = []
    for c in range(num_devices):
        g, r = c // 4, c % 4
        idx = stripe_idx(r)
        x0 = inp["input"][g].astype(f32)
        m = {
            "x0T_b": np.ascontiguousarray(x0.T).astype(bf),
            "x0rT_b": np.ascontiguousarray(x0[idx].T).astype(bf),
            "x0res": (x0[idx] + inp["sa_bo"][None, :]).astype(f32),
            "encT_b": np.ascontiguousarray(
                inp["encoder_output"][g].T).astype(bf),
            "ca_bo": inp["ca_bo"].astype(bf),
            "w1": inp["ffn_w1"].astype(bf),
            "b1": inp["ffn_b1"].astype(f32),
            "w2": inp["ffn_w2"].astype(bf),
            "b2": inp["ffn_b2"].astype(bf),
            "cmask4": make_cmask4(r),
        }
        for p in ("sa", "ca"):
            m[f"{p}_wkv"] = np.concatenate(
                [inp[f"{p}_wk"], inp[f"{p}_wv"]], axis=1).astype(bf)
            m[f"{p}_wq"] = inp[f"{p}_wq"].astype(bf)
            m[f"{p}_wo"] = inp[f"{p}_wo"].astype(bf)
            m[f"{p}_bq"] = inp[f"{p}_bq"].astype(f32)
            m[f"{p}_bk"] = inp[f"{p}_bk"].astype(f32)
            m[f"{p}_bv"] = inp[f"{p}_bv"].astype(bf)
        for i in (1, 2, 3):
            m[f"ln{i}_g"] = inp[f"ln{i}_g"].astype(bf)
            m[f"ln{i}_b"] = inp[f"ln{i}_b"].astype(bf)
        in_maps.append(m)
    return in_maps


def unshard_outputs(per_core, B=2):
    """per_core: list/array of 8 x [SQ, E] -> [B, S, E]."""
    full = np.zeros((B, S, E), dtype=np.float32)
    for c in range(8):
        g, r = c // 4, c % 4
        full[g, stripe_idx(r)] = np.asarray(per_core[c], dtype=np.float32)
    return full


_NC_CACHE = {}


def _get_nc(S_arg):
    if S_arg not in _NC_CACHE:
        _NC_CACHE[S_arg] = build_decoder_nc(S_arg)
    return _NC_CACHE[S_arg]


def kernel(**inputs):
    x = np.asarray(inputs["input"])
    B, S_arg, _ = x.shape
    nc = _get_nc(S_arg)
    in_maps = shard_inputs(inputs)
    res = bass_utils.run_bass_kernel_spmd(nc, in_maps, core_ids=list(range(8)))
    return unshard_outputs([res.results[c]["out"] for c in range(8)], B=B)



# revision 71
# speedup vs baseline: 1.1645x; 1.1645x over previous
"""Transformer decoder layer (causal self-attn + cross-attn + FFN, 3 post-LNs)
on 8 Trainium2 NeuronCores — token-parallel version, zero collectives.

Sharding: 2-way data parallel (batch) x 4-way query-token striping.
  core c: batch g = c // 4, stripe r = c % 4 owns the 128-row blocks
  {r, r+4, r+8, r+12} of the sequence (512 query tokens).
  - K/V are computed redundantly on every core from the full input /
    encoder_output (which each core holds) for all 16 heads.
  - out-projections and the FFN are complete per token -> no reductions.
  - causality is data-driven (cmask4 per core), so the instruction
    stream is identical on all cores (true SPMD).

Precision (validated at rel_l2 ~3.4e-3 vs the f32 reference):
  - attention path runs fp8e4m3 with DoubleRow matmuls (2 k-tiles per
    instruction): activations x/enc/x1 in fp8, wq/wk/wv/wo stored x16 in
    fp8 (descaled by 1/16 in the PSUM->SBUF bias ops), V and exp(scores)
    and oT in fp8;
  - the FFN (x2, w1, h, w2) stays bf16 — w1/w2/x2 quantization each cost
    ~1.3e-2 output error (no averaging downstream), everything else <2e-3.
  - qk bias stays exact; V bias is folded into the out-proj bias host-side
    (exact: sum a/Z = 1), so the V PSUM->SBUF move is a pure scaled copy.

On-chip layouts (per core):
  x0T/encT  [128, 8, 2048]  fp8    feature-major full activations
  x0rT/x1rT [128, 8, 512]   fp8    feature-major own-token activations
  x2rT      [128, 8, 512]   bf16   (FFN input, kept bf16)
  kT        [128, 8, 2048]  bf16   head-dim on partitions (2 heads x 64)
  qT        [128, 8, 512]   bf16
  v         [128, 16, 16, 65] fp8  token-major V (+ ones col = rowsum)
  at2       [128, 2, 2, 512] fp8   exp(scores) per kb-pair (pipelined)
  pot       [128, 2, 512] PSUM f32 v.T @ at per head-pair (row 64 = Z)
  oT        [128, 8, 512]   fp8    normalized attention out, feature-major

Schedule notes (HW-measured):
  - attention is Act-engine(exp)-bound; everything else must stay out of
    the Act queue (PSUM-reading Act ops ahead of exps cost ~100+ us HW).
  - softmax 1/Z normalize is deferred one head-pair so the PE queue never
    stalls on the reciprocal chain.
  - CA V-projection groups are interleaved into the SA out-proj/LN1
    window; kT groups for heads 2..7 are emitted just-in-time.
"""

import numpy as np
import ml_dtypes

import concourse.bass as bass
import concourse.bacc as bacc
import concourse.tile as tile
from concourse import mybir
from concourse import bass_utils
from concourse.masks import make_identity

F32 = mybir.dt.float32
BF16 = mybir.dt.bfloat16
FP8 = mybir.dt.float8e4
AF = mybir.ActivationFunctionType
ALU = mybir.AluOpType
DR = mybir.MatmulPerfMode.DoubleRow

WS = 16.0          # fp8 weight scale (weights stored x16, descaled on-chip)
RWS = 1.0 / WS

# tuning knobs (sim-swept)
ATTN_KT_INTERLEAVE = False   # trickle next attention's kT groups into pairs
MASK_ENGINES = ("vector",)            # alternate per kb
KT_BIAS_ENGINES = ("vector",)         # engines for kT bias-add, cycled
NORM_BCAST = "pe"                     # "gpsimd" | "pe" 1/Z row broadcast

E = 1024
EB = 8           # E / 128
H = 16
HP = 8           # head pairs
DK = 64
S = 2048
TB = 16          # full-token 128-blocks
TBQ = 4          # own-token 128-blocks
SQ = 512         # own query tokens


def _ts(i, n):
    return slice(i * n, (i + 1) * n)


def _pbcast(ap, p=128):
    """Broadcast a 1D DRAM AP across p partitions (partition step 0)."""
    return bass.AP(tensor=ap.tensor, offset=ap.offset, ap=[[0, p]] + list(ap.ap))


PHASES = ["null", "x0t", "saqkv", "saattn", "ln1", "cakv", "caattn",
          "ln2", "ffn1", "ffn2", "full"]


def build_decoder_nc(S_arg: int = S, num_devices: int = 8,
                     stop_after: str | None = None, reps: int = 1):
    assert S_arg == S
    nc = bacc.Bacc("TRN2", target_bir_lowering=False, debug=False,
                   num_devices=num_devices)

    din = {}

    def inp(name, shape, dt):
        din[name] = nc.dram_tensor(name, list(shape), dt, kind="ExternalInput")
        return din[name]

    inp("x0T_b", [E, S], FP8)            # input (batch g), host-transposed
    inp("x0rT_b", [E, SQ], FP8)          # own stripes, host-transposed
    inp("x0res", [SQ, E], F32)           # own stripes + sa_bo (residual)
    inp("encT_b", [E, S], FP8)           # encoder out, host-transposed
    for p in ("sa", "ca"):
        inp(f"{p}_wkv", [E, 2 * E], FP8)     # [wk | wv] x WS
        inp(f"{p}_wq", [E, E], FP8)          # x WS
        inp(f"{p}_wo", [E, E], FP8)          # x WS
        inp(f"{p}_bq", [E], F32)
        inp(f"{p}_bk", [E], F32)
    inp("ca_bo", [E], BF16)              # ca_bo + ca_bv @ ca_wo (host-folded)
    inp("w1", [E, 4 * E], BF16)
    inp("b1", [4 * E], F32)
    inp("w2", [4 * E, E], BF16)
    inp("b2", [E], BF16)
    for i in (1, 2, 3):
        inp(f"ln{i}_g", [E], BF16)
        inp(f"ln{i}_b", [E], BF16)
    inp("cmask4", [4, 128, 128], FP8)    # per-core causal block masks

    out = nc.dram_tensor("out", [SQ, E], F32, kind="ExternalOutput")

    with tile.TileContext(nc) as tc:
        _emit(tc, din, out, stop_after, reps=reps)

    nc.compile()
    return nc


def _emit(tc, din, out, stop_after=None, reps=1):
    nc = tc.nc

    def cut(phase):
        return stop_after == phase

    def finish():
        nc.sync.dma_start(out=out.ap(), in_=din["x0res"].ap())

    from contextlib import ExitStack
    with ExitStack() as _es:
        _es.enter_context(nc.allow_low_precision(
            reason="fp8 attn weights / bf16 softmax rcp; validated by rel-err"))
        def _pool(**kw):
            return _es.enter_context(tc.tile_pool(**kw))
        const = _pool(name="const", bufs=1)
        xt_pool = _pool(name="xt", bufs=1)          # x0T / encT
        wkv_pool = _pool(name="wkv", bufs=1)        # wkv / w1-half
        wq_pool = _pool(name="wq", bufs=1)          # wq / wo
        kt_pool = _pool(name="kt", bufs=1)          # kT / w2-half
        v_pool = _pool(name="vp", bufs=1)           # v / ffn y
        qt_pool = _pool(name="qt", bufs=1)
        attn_pool = _pool(name="att", bufs=2)       # at / hT
        o_pool = _pool(name="ot", bufs=1)
        xrt_pool = _pool(name="xrt", bufs=1)        # xrowT
        res_pool = _pool(name="res", bufs=1)        # x1/x2 row bf16
        lnp = _pool(name="lnp", bufs=4)
        lnr = _pool(name="lnr", bufs=1)
        zp = _pool(name="zp", bufs=2)
        stat = _pool(name="stat", bufs=8)
        mmp = _pool(name="mmp", bufs=2, space="PSUM")
        po_pool = _pool(name="po", bufs=2, space="PSUM")
        # ---------------- constants ----------------
        ident = const.tile([128, 128], BF16)
        make_identity(nc, ident)
        eps_t = const.tile([128, 1], F32)
        nc.vector.memset(eps_t, 1e-12)
        ones64 = const.tile([1, 64], BF16)
        nc.vector.memset(ones64, 1.0)
        cmask4 = const.tile([128, 4, 128], FP8)
        nc.sync.dma_start(out=cmask4,
                          in_=din["cmask4"].ap().rearrange("i p q -> p i q"))

        _bc = {}

        def bcast(name, tag=None, width=E):
            if name not in _bc:
                t = const.tile([128, width], BF16, name=f"bc_{name}",
                               tag=tag or f"bc_{name}")
                nc.sync.dma_start(out=t, in_=_pbcast(din[name].ap()))
                _bc[name] = t
            return _bc[name]

        def pp_bias(name, nj, dt=F32):
            t = const.tile([128, nj], dt, name=f"ppb_{name}")
            nc.sync.dma_start(out=t,
                              in_=din[name].ap().rearrange("(j p) -> p j", p=128))
            return t

        bq = {p: pp_bias(f"{p}_bq", 8) for p in ("sa", "ca")}
        bk = {p: pp_bias(f"{p}_bk", 8) for p in ("sa", "ca")}
        b1_t = pp_bias("b1", 32)

        # ---------------- shared helpers ----------------
        def load_wkv(pref):
            t = wkv_pool.tile([128, EB, 2 * E], FP8, tag="wkv",
                              name=f"{pref}_wkv")
            nc.sync.dma_start(out=t, in_=din[f"{pref}_wkv"].ap().rearrange(
                "(eb p) m -> p eb m", p=128))
            return t

        def load_sq(name, tag="wq"):
            t = wq_pool.tile([128, EB, E], FP8, tag=tag, name=f"{name}_sb")
            nc.sync.dma_start(out=t, in_=din[name].ap().rearrange(
                "(eb p) m -> p eb m", p=128))
            return t

        def kt_steps(xT, wkv, bkt, kT, hps, engines=None):
            # kT [128, hp, S] bf16; one yield per (hp, t2) group
            engines = engines or KT_BIAS_ENGINES
            g = 0
            for hp in hps:
                for t2 in range(2):
                    mm = mmp.tile([128, 1024], F32, tag="mm")
                    for i in range(2):
                        tt = 2 * t2 + i
                        for e2 in range(EB // 2):
                            nc.tensor.matmul(mm[:, _ts(i, 512)],
                                             wkv[:, 2 * e2:2 * e2 + 2,
                                                 _ts(hp, 128)],
                                             xT[:, 2 * e2:2 * e2 + 2,
                                                _ts(tt, 512)],
                                             start=(e2 == 0),
                                             stop=(e2 == EB // 2 - 1),
                                             perf_mode=DR)
                    eng = engines[g % len(engines)]
                    g += 1
                    if eng == "scalar":
                        nc.scalar.activation(kT[:, hp, _ts(t2, 1024)], mm,
                                             AF.Identity,
                                             bias=bkt[:, hp:hp + 1],
                                             scale=RWS)
                    else:
                        nc.vector.tensor_scalar(kT[:, hp, _ts(t2, 1024)], mm,
                                                RWS, bkt[:, hp:hp + 1],
                                                ALU.mult, ALU.add)
                    yield

        def v_steps(xT, wkv, v, on_act=True):
            # v [128, tb, h, 65] fp8; one yield per tb group.
            # v bias is folded into the out-proj bias host-side, so the
            # PSUM->SBUF move is a pure scaled copy.
            for tb in range(TB):
                mm = mmp.tile([128, 1024], F32, tag="mm")
                for half in range(2):
                    for e2 in range(EB // 2):
                        nc.tensor.matmul(mm[:, _ts(half, 512)],
                                         xT[:, 2 * e2:2 * e2 + 2, _ts(tb, 128)],
                                         wkv[:, 2 * e2:2 * e2 + 2,
                                             E + half * 512:
                                             E + half * 512 + 512],
                                         start=(e2 == 0),
                                         stop=(e2 == EB // 2 - 1),
                                         perf_mode=DR)
                act_now = on_act if on_act != "mix" else (tb % 2 == 0)
                if act_now:
                    nc.scalar.activation(
                        v[:, tb, :, 0:64],
                        mm.rearrange("p (h d) -> p h d", d=64),
                        AF.Copy, scale=RWS)
                else:
                    nc.vector.tensor_scalar_mul(
                        v[:, tb, :, 0:64],
                        mm.rearrange("p (h d) -> p h d", d=64), RWS)
                yield

        def drain(gen):
            for _ in gen:
                pass

        def q_proj(xrT, wq, bqt, qT):
            for hp2 in range(HP // 2):
                mm = mmp.tile([128, 1024], F32, tag="mm")
                for i in range(2):
                    hp = 2 * hp2 + i
                    for e2 in range(EB // 2):
                        nc.tensor.matmul(mm[:, _ts(i, 512)],
                                         wq[:, 2 * e2:2 * e2 + 2, _ts(hp, 128)],
                                         xrT[:, 2 * e2:2 * e2 + 2, :],
                                         start=(e2 == 0),
                                         stop=(e2 == EB // 2 - 1),
                                         perf_mode=DR)
                for i in range(2):
                    nc.vector.tensor_scalar(qT[:, 2 * hp2 + i, :],
                                            mm[:, _ts(i, 512)], RWS,
                                            bqt[:, 2 * hp2 + i:2 * hp2 + i + 1],
                                            ALU.mult, ALU.add)

        def attention(qT, kT, v, oT, causal, interleave=None):
            NP = TB // 2    # kb pairs

            def fill(n=1):
                if interleave is None:
                    return
                for _ in range(n):
                    try:
                        next(interleave)
                    except StopIteration:
                        return

            def normalize(hp, pot):
                for h2 in range(2):
                    hs = slice(h2 * 64, h2 * 64 + 64)
                    rzb = zp.tile([1, 512], BF16, tag="zb")
                    nc.vector.reciprocal(rzb, pot[64:65, h2, :])
                    pbs = zp.tile([64, 512], BF16, tag="pbs")
                    if NORM_BCAST == "gpsimd":
                        # broadcast 1/Z down 64 partitions on gpsimd
                        nc.gpsimd.partition_broadcast(pbs, rzb)
                    else:
                        # rank-1 matmul broadcast; deferred emission keeps the
                        # reciprocal chain off the PE queue's critical path
                        pb = mmp.tile([128, 1024], F32, tag="mm")
                        nc.tensor.matmul(pb[0:64, 0:512], ones64, rzb,
                                         start=True, stop=True)
                        nc.vector.tensor_copy(pbs, pb[0:64, 0:512])
                    nc.vector.tensor_tensor(oT[hs, hp, :], pot[0:64, h2, :],
                                            pbs, ALU.mult)

            prev = None
            for hp in range(HP):
                pot = po_pool.tile([128, 2, 512], F32, tag="po")
                for pair in range(NP):
                    # at2 [p, kb2, h2, q] for this kb pair only
                    at2 = attn_pool.tile([128, 2, 2, 512], FP8, tag="at")
                    c0p = 128 * (2 * pair // 4) if causal else 0
                    for i in range(2):
                        kb = 2 * pair + i
                        c0 = 128 * (kb // 4) if causal else 0
                        mm = mmp.tile([128, 1024], F32, tag="mm")
                        mm2 = mm.rearrange("p (i q) -> p i q", i=2)
                        nc.tensor.matmul(mm[:, c0:512],
                                         kT[0:64, hp, _ts(kb, 128)],
                                         qT[0:64, hp, c0:], start=True,
                                         stop=True, tile_position=(0, 0))
                        nc.tensor.matmul(mm[:, 512 + c0:],
                                         kT[64:128, hp, _ts(kb, 128)],
                                         qT[64:128, hp, c0:], start=True,
                                         stop=True, tile_position=(64, 0))
                        nc.scalar.activation(at2[:, i, :, c0:], mm2[:, :, c0:],
                                             AF.Exp, scale=0.125)
                        if causal:
                            cm = cmask4[:, kb % 4, :]
                            cm2 = bass.AP(tensor=cm.tensor, offset=cm.offset,
                                          ap=[list(cm.ap)[0], [0, 2],
                                              list(cm.ap)[1]])
                            eng = getattr(
                                nc, MASK_ENGINES[kb % len(MASK_ENGINES)])
                            eng.tensor_mul(at2[:, i, :, c0:c0 + 128],
                                           at2[:, i, :, c0:c0 + 128],
                                           cm2)
                    fill()
                    for h2 in range(2):
                        nc.tensor.matmul(pot[0:65, h2, c0p:],
                                         v[:, 2 * pair:2 * pair + 2,
                                           2 * hp + h2, :],
                                         at2[:, :, h2, c0p:],
                                         start=(pair == 0),
                                         stop=(pair == NP - 1),
                                         perf_mode=DR,
                                         skip_group_check=True)
                    if pair == 1 and prev is not None:
                        # deferred: previous hp's softmax scale, emitted after
                        # this hp's first scores so the PE queue never stalls
                        # on the reciprocal chain
                        normalize(*prev)
                        prev = None
                if prev is not None:
                    normalize(*prev)
                prev = (hp, pot)
            normalize(*prev)

        def layer_norm_block(ld, i, out_bf, tb):
            # in-place LN of ld [128, E]; writes bf16 copy to out_bf[:, tb, :]
            st = stat.tile([128, 2, 6], F32, tag="bnst")
            for sg in range(2):
                nc.vector.bn_stats(st[:, sg, :], ld[:, _ts(sg, 512)])
            mv = stat.tile([128, 2], F32, tag="bnmv")
            nc.vector.bn_aggr(mv, st)
            sd = stat.tile([128, 1], F32, tag="sd")
            nc.scalar.activation(sd, mv[:, 1:2], AF.Sqrt, bias=eps_t)
            rstd = stat.tile([128, 1], F32, tag="rstd")
            nc.vector.reciprocal(rstd, sd)
            if i > 1:
                # Act is idle after CA attention: (x - m)*rstd as
                # Identity(x*rstd + (-m*rstd)) to shorten the DVE chain
                nmr = stat.tile([128, 1], F32, tag="nmr")
                nc.vector.tensor_scalar(nmr, mv[:, 0:1], rstd, -1.0,
                                        ALU.mult, ALU.mult)
                nc.scalar.activation(ld, ld, AF.Identity, bias=nmr,
                                     scale=rstd)
            else:
                nc.vector.tensor_scalar(ld, ld, mv[:, 0:1], rstd,
                                        ALU.subtract, ALU.mult)
            nc.vector.tensor_mul(ld, ld, bcast(f"ln{i}_g"))
            if out_bf is not None:
                nc.vector.tensor_add(out_bf[:, tb, :], ld, bcast(f"ln{i}_b"))
            return ld

        def out_proj_ln(oT, wo, i, residual, out_bf, interleave=None):
            # y = oT.T @ wo (+ residual [+ca_bo]) -> LN_i -> out_bf bf16
            def fill(n):
                if interleave is None:
                    return
                for _ in range(n):
                    try:
                        next(interleave)
                    except StopIteration:
                        return

            for tb in range(TBQ):
                ld = lnp.tile([128, E], F32, tag="ln_io")
                if i == 1:
                    res = lnr.tile([128, E], F32, tag="ln_res")
                    nc.gpsimd.dma_start(out=res,
                                        in_=din["x0res"].ap()[_ts(tb, 128), :])
                mm = mmp.tile([128, 1024], F32, tag="mm")
                for ns in range(2):
                    for j2 in range(EB // 2):
                        nc.tensor.matmul(mm[:, _ts(ns, 512)],
                                         oT[:, 2 * j2:2 * j2 + 2, _ts(tb, 128)],
                                         wo[:, 2 * j2:2 * j2 + 2, _ts(ns, 512)],
                                         start=(j2 == 0),
                                         stop=(j2 == EB // 2 - 1),
                                         perf_mode=DR)
                if i == 1:
                    nc.vector.scalar_tensor_tensor(ld, mm, RWS, res,
                                                   ALU.mult, ALU.add)
                else:
                    nc.vector.scalar_tensor_tensor(
                        ld, mm, RWS, bcast("ca_bo", tag="bob"),
                        ALU.mult, ALU.add)
                    nc.vector.tensor_add(ld, ld, residual[:, tb, :])
                layer_norm_block(ld, i, out_bf, tb)
                fill(4)

        def row_transpose(src_bf, dst_T, interleave=None):
            # src [128, TBQ, E] bf16 token-major -> dst [128, EB, SQ]
            for tb in range(TBQ):
                for eb in range(EB):
                    pt = mmp.tile([128, 1024], BF16, tag="mm")
                    nc.tensor.transpose(pt[:, 0:128], src_bf[:, tb, _ts(eb, 128)],
                                        ident)
                    nc.vector.tensor_copy(dst_T[:, eb, _ts(tb, 128)], pt[:, 0:128])
                if interleave is not None:
                    try:
                        next(interleave)
                    except StopIteration:
                        interleave = None

        # ================= start =================
        if cut("null"):
            finish()
            return
        for _rep in range(reps):

            x0T = xt_pool.tile([128, EB, S], FP8, tag="xT", name="x0T")
            nc.gpsimd.dma_start(out=x0T, in_=din["x0T_b"].ap().rearrange(
                "(eb p) t -> p eb t", p=128))
            x0rT = xrt_pool.tile([128, EB, SQ], FP8, tag="xrT", name="x0rT")
            nc.sync.dma_start(out=x0rT, in_=din["x0rT_b"].ap().rearrange(
                "(eb p) t -> p eb t", p=128))

            if cut("x0t"):
                finish()
                return

            sa_wkv = load_wkv("sa")
            sa_wq = load_sq("sa_wq")

            kT = kt_pool.tile([128, HP, S], BF16, tag="kT", name="sa_kT")
            v = v_pool.tile([128, TB, H, 65], FP8, tag="v", name="sa_v")
            nc.vector.memset(v[:, :, :, 64:65], 1.0)
            qT = qt_pool.tile([128, HP, SQ], BF16, tag="qT", name="sa_qT")

            q_proj(x0rT, sa_wq, bq["sa"], qT)
            drain(kt_steps(x0T, sa_wkv, bk["sa"], kT, [0, 1]))
            drain(v_steps(x0T, sa_wkv, v, on_act=False))

            # loads that overlap SA attention (slots freed by the projections)
            sa_wo = load_sq("sa_wo")
            encT = xt_pool.tile([128, EB, S], FP8, tag="xT", name="encT")
            nc.gpsimd.dma_start(out=encT, in_=din["encT_b"].ap().rearrange(
                "(eb p) t -> p eb t", p=128))
            ca_wkv = load_wkv("ca")

            if cut("saqkv"):
                finish()
                return

            sa_ktgen = kt_steps(x0T, sa_wkv, bk["sa"], kT, range(2, HP))
            if not ATTN_KT_INTERLEAVE:
                drain(sa_ktgen)
                sa_ktgen = None
            oT = o_pool.tile([128, HP, SQ], FP8, tag="oT", name="sa_oT")
            attention(qT, kT, v, oT, causal=True, interleave=sa_ktgen)

            if cut("saattn"):
                finish()
                return

            # ================= cross-attention =================
            ca_v = v_pool.tile([128, TB, H, 65], FP8, tag="v", name="ca_v")
            nc.vector.memset(ca_v[:, :, :, 64:65], 1.0)
            ca_kT = kt_pool.tile([128, HP, S], BF16, tag="kT", name="ca_kT")
            ca_vgen = v_steps(encT, ca_wkv, ca_v, on_act=True)

            x1row_bf = res_pool.tile([128, TBQ, E], BF16, tag="res", name="xrow_bf")
            out_proj_ln(oT, sa_wo, 1, None, x1row_bf, interleave=ca_vgen)

            ca_wq = load_sq("ca_wq")

            if cut("ln1"):
                finish()
                return

            x1rT = xrt_pool.tile([128, EB, SQ], FP8, tag="xrT", name="x1rT")
            row_transpose(x1row_bf, x1rT, interleave=ca_vgen)
            drain(ca_vgen)
            drain(kt_steps(encT, ca_wkv, bk["ca"], ca_kT, [0, 1],
                           engines=("vector", "scalar")))

            ca_qT = qt_pool.tile([128, HP, SQ], BF16, tag="qT", name="ca_qT")
            q_proj(x1rT, ca_wq, bq["ca"], ca_qT)

            ca_wo = load_sq("ca_wo")

            if cut("cakv"):
                finish()
                return

            ca_ktgen = kt_steps(encT, ca_wkv, bk["ca"], ca_kT, range(2, HP),
                                engines=("vector", "scalar"))
            if not ATTN_KT_INTERLEAVE:
                drain(ca_ktgen)
                ca_ktgen = None
            ca_oT = o_pool.tile([128, HP, SQ], FP8, tag="oT", name="ca_oT")
            attention(ca_qT, ca_kT, ca_v, ca_oT, causal=False,
                      interleave=ca_ktgen)

            # FFN pass-0 weight prefetch (overlaps CA out-proj/LN2)
            w1ap = din["w1"].ap().rearrange("(eb p) m -> p eb m", p=128)
            w1p0 = wkv_pool.tile([128, EB, 2 * E], BF16, tag="wkv", name="w1_0")
            nc.sync.dma_start(out=w1p0, in_=w1ap[:, :, _ts(0, 2 * E)])
            w2p0 = kt_pool.tile([128, 16, E], BF16, tag="kT", name="w2_0")
            nc.sync.dma_start(
                out=w2p0,
                in_=din["w2"].ap()[_ts(0, 2 * E), :].rearrange(
                    "(fb q) n -> q fb n", q=128))

            if cut("caattn"):
                finish()
                return

            out_proj_ln(ca_oT, ca_wo, 2, x1row_bf, x1row_bf)  # x2row overwrites

            if cut("ln2"):
                finish()
                return

            # ================= FFN =================
            x2rT = xrt_pool.tile([128, EB, SQ], BF16, tag="xrT", name="x2rT")
            row_transpose(x1row_bf, x2rT)

            y = v_pool.tile([128, TBQ, E], F32, tag="v", name="ffn_y")
            for p in range(2):
                if p == 0:
                    w1p, w2p = w1p0, w2p0
                else:
                    w1p = wkv_pool.tile([128, EB, 2 * E], BF16, tag="wkv",
                                        name=f"w1_{p}")
                    nc.sync.dma_start(out=w1p, in_=w1ap[:, :, _ts(p, 2 * E)])
                    w2p = kt_pool.tile([128, 16, E], BF16, tag="kT",
                                       name=f"w2_{p}")
                    nc.sync.dma_start(
                        out=w2p,
                        in_=din["w2"].ap()[_ts(p, 2 * E), :].rearrange(
                            "(fb q) n -> q fb n", q=128))
                hT = attn_pool.tile([128, 16, 512], BF16, tag="at",
                                    name=f"hT_{p}")
                for hb2 in range(8):
                    mm = mmp.tile([128, 1024], F32, tag="mm")
                    for i in range(2):
                        hb = 2 * hb2 + i
                        for eb in range(EB):
                            nc.tensor.matmul(mm[:, _ts(i, 512)],
                                             w1p[:, eb, _ts(hb, 128)],
                                             x2rT[:, eb, :],
                                             start=(eb == 0),
                                             stop=(eb == EB - 1))
                    for i in range(2):
                        hb = 2 * hb2 + i
                        gfb = p * 16 + hb
                        # relu on DVE: (mm + b1) then max(0, .) — keeps the
                        # Act engine free for attention exps
                        nc.vector.tensor_scalar(hT[:, hb, :], mm[:, _ts(i, 512)],
                                                b1_t[:, gfb:gfb + 1], 0.0,
                                                ALU.add, ALU.max)
                if cut("ffn1") and p == 0:
                    finish()
                    return
                for tb in range(TBQ):
                    mm = mmp.tile([128, 1024], F32, tag="mm")
                    for ns in range(2):
                        for fb in range(16):
                            nc.tensor.matmul(mm[:, _ts(ns, 512)],
                                             hT[:, fb, _ts(tb, 128)],
                                             w2p[:, fb, _ts(ns, 512)],
                                             start=(fb == 0), stop=(fb == 15))
                    if p == 0:
                        nc.vector.scalar_tensor_tensor(
                            y[:, tb, :], mm, 1.0, bcast("b2", tag="bob"),
                            ALU.mult, ALU.add)
                    else:
                        ld = lnp.tile([128, E], F32, tag="ln_io")
                        nc.vector.tensor_tensor(ld, y[:, tb, :], mm, ALU.add)
                        nc.vector.tensor_add(ld, ld, x1row_bf[:, tb, :])
                        layer_norm_block(ld, 3, None, tb)
                        nc.vector.tensor_add(ld, ld, bcast("ln3_b"))
                        nc.sync.dma_start(out=out.ap()[_ts(tb, 128), :], in_=ld)

        if cut("ffn2"):
            return


# ====================== host side ======================

def stripe_idx(r):
    blocks = [r, r + 4, r + 8, r + 12]
    return np.concatenate([np.arange(128 * b, 128 * b + 128) for b in blocks])


def make_cmask4(r):
    # i < r: pass; i == r: lower-tri straddle; i > r: blocked
    m = np.zeros((4, 128, 128), dtype=np.float32)
    pk = np.arange(128)[:, None]
    pq = np.arange(128)[None, :]
    for i in range(4):
        if i < r:
            m[i] = 1.0
        elif i == r:
            m[i] = (pk <= pq).astype(np.float32)
    return m.astype(ml_dtypes.float8_e4m3fn)


def shard_inputs(inputs, num_devices=8):
    bf = ml_dtypes.bfloat16
    f8 = ml_dtypes.float8_e4m3fn
    f32 = np.float32
    inp = {k: np.asarray(v) for k, v in inputs.items()}
    in_maps = []
    for c in range(num_devices):
        g, r = c // 4, c % 4
        idx = stripe_idx(r)
        x0 = inp["input"][g].astype(f32)
        m = {
            "x0T_b": np.ascontiguousarray(x0.T).astype(f8),
            "x0rT_b": np.ascontiguousarray(x0[idx].T).astype(f8),
            "x0res": (x0[idx] + (inp["sa_bo"] + inp["sa_bv"] @ inp["sa_wo"]
                                 )[None, :]).astype(f32),
            "encT_b": np.ascontiguousarray(
                inp["encoder_output"][g].T).astype(f8),
            "ca_bo": (inp["ca_bo"] + inp["ca_bv"] @ inp["ca_wo"]).astype(bf),
            "w1": inp["ffn_w1"].astype(bf),
            "b1": inp["ffn_b1"].astype(f32),
            "w2": inp["ffn_w2"].astype(bf),
            "b2": inp["ffn_b2"].astype(bf),
            "cmask4": make_cmask4(r),
        }
        for p in ("sa", "ca"):
            m[f"{p}_wkv"] = (np.concatenate(
                [inp[f"{p}_wk"], inp[f"{p}_wv"]], axis=1) * WS).astype(f8)
            m[f"{p}_wq"] = (inp[f"{p}_wq"] * WS).astype(f8)
            m[f"{p}_wo"] = (inp[f"{p}_wo"] * WS).astype(f8)
            m[f"{p}_bq"] = inp[f"{p}_bq"].astype(f32)
            m[f"{p}_bk"] = inp[f"{p}_bk"].astype(f32)
        for i in (1, 2, 3):
            m[f"ln{i}_g"] = inp[f"ln{i}_g"].astype(bf)
            m[f"ln{i}_b"] = inp[f"ln{i}_b"].astype(bf)
        in_maps.append(m)
    return in_maps


def unshard_outputs(per_core, B=2):
    """per_core: list/array of 8 x [SQ, E] -> [B, S, E]."""
    full = np.zeros((B, S, E), dtype=np.float32)
    for c in range(8):
        g, r = c // 4, c % 4
        full[g, stripe_idx(r)] = np.asarray(per_core[c], dtype=np.float32)
    return full


_NC_CACHE = {}


def _get_nc(S_arg):
    if S_arg not in _NC_CACHE:
        _NC_CACHE[S_arg] = build_decoder_nc(S_arg)
    return _NC_CACHE[S_arg]


def kernel(**inputs):
    x = np.asarray(inputs["input"])
    B, S_arg, _ = x.shape
    nc = _get_nc(S_arg)
    in_maps = shard_inputs(inputs)
    res = bass_utils.run_bass_kernel_spmd(nc, in_maps, core_ids=list(range(8)))
    return unshard_outputs([res.results[c]["out"] for c in range(8)], B=B)

